# revision 1
# baseline (speedup 1.0000x reference)
"""CAM (channel attention) module kernel for 8 TRN2 NeuronCores.

Reference computation (per batch b of 32, C=2048, N=H*W=196):
    E = q @ q.T                      # [C, C] channel energy
    A = softmax(rowmax(E) - E)       # == softmax(-E) row-wise (shift cancels)
    out = gamma * (A @ q) + x
    y = conv1x1(out, W) + bias       # 2048 -> 512

Sharding: pure data-parallel over batch, 4 batches per core, conv weights
replicated. No collectives.

Per-core kernel design (symmetric-E):
  - E = q q^T is symmetric: compute only block-upper-triangle (row-tile j
    covers columns [256*(j//2), 2048) in bank-aligned 512/256-wide fp32r
    (FP22 tensor-engine mode) matmuls, K padded 196->2x128.
  - U = exp(-E - 40) via ScalarE activation straight out of PSUM into bf16
    (constant shift keeps exp in fp32 range for N(0,1) inputs of this size;
    any constant cancels in the softmax normalization).
  - Lower-triangle U blocks are mirror-filled with batched SBUF->SBUF DMA
    block-transposes (bf16 XBAR path, one 3D-out instruction per source
    row-tile, issued on the sync queue; off-engine).
  - U symmetric => stored U tiles serve directly as lhsT for O = U @ q
    (bf16, free dim 197: q plus a ones column that yields the row sums Z).
  - normalize + residual: xout = O[:, :196] * (gamma/Z) + x on VectorE.
  - 1x1 conv as matmul over C with batch-pair-concatenated free dim (392)
    in float32r, bias added on the PSUM->SBUF evacuation.
"""

import numpy as np

B = 32
NCORES = 8
BL = B // NCORES  # batches per core
C = 2048
HW = 196
OUT = 512
CT = C // 128  # 16 c-tiles
SHIFT = 40.0

_CACHE = {}


def _build_nc(reps=1):
    import contextlib
    import concourse.bacc as bacc
    import concourse.tile as tile
    import concourse.mybir as mybir

    f32 = mybir.dt.float32
    f32r = mybir.dt.float32r
    bf16 = mybir.dt.bfloat16
    FT = mybir.ActivationFunctionType
    ALU = mybir.AluOpType

    nc = bacc.Bacc("TRN2", target_bir_lowering=False, debug=False,
                   num_devices=NCORES)

    qTd = nc.dram_tensor("qT", [BL, 2, 128, C], f32r, kind="ExternalInput")
    qnd = nc.dram_tensor("qn", [BL, CT, 128, HW], f32, kind="ExternalInput")
    wTd = nc.dram_tensor("wT", [CT, 128, OUT], f32r, kind="ExternalInput")
    biasd = nc.dram_tensor("bias", [128, OUT // 128], f32, kind="ExternalInput")
    gammad = nc.dram_tensor("gammac", [128, 1], f32, kind="ExternalInput")
    outd = nc.dram_tensor("out", [BL, OUT, HW], f32, kind="ExternalOutput")

    with tile.TileContext(nc) as tc:
        with (
            tc.tile_pool(name="const", bufs=1) as constp,
            tc.tile_pool(name="qt", bufs=2) as qtp,
            tc.tile_pool(name="qbp", bufs=2) as qbp,
            tc.tile_pool(name="qnp", bufs=2) as qnp,
            tc.tile_pool(name="u", bufs=1) as up,
            tc.tile_pool(name="xo", bufs=1) as xop,
            tc.tile_pool(name="y", bufs=4) as yp,
            tc.tile_pool(name="z", bufs=8) as zp,
            tc.tile_pool(name="psum", bufs=2, space="PSUM") as psp,
        ):
            wT_s = constp.tile([128, CT, OUT], f32r)
            nc.sync.dma_start(wT_s[:], wTd[:].rearrange("i p o -> p i o"))
            bias_s = constp.tile([128, OUT // 128], f32)
            nc.sync.dma_start(bias_s[:], biasd[:])
            gamma_s = constp.tile([128, 1], f32)
            nc.sync.dma_start(gamma_s[:], gammad[:])
            shift_s = constp.tile([128, 1], f32)
            nc.vector.memset(shift_s[:], -SHIFT)
            ones_b = constp.tile([128, 1], bf16)
            nc.vector.memset(ones_b[:], 1.0)

            U = up.tile([128, CT, C], bf16)

            rep_ctx = (
                tc.For_i(0, reps, 1,
                         hint_engines=tuple(mybir.EngineType))
                if reps > 1 else contextlib.nullcontext()
            )
            def load_batch(b):
                qT_s = qtp.tile([128, 2, C], f32r, tag="qt")
                nc.sync.dma_start(qT_s[:, 0], qTd[b, 0])
                nc.sync.dma_start(qT_s[:, 1], qTd[b, 1])
                qn_s = qnp.tile([128, CT, HW], f32, tag="qn")
                nc.sync.dma_start(qn_s[:], qnd[b].rearrange("i p n -> p i n"))
                qb_s = qbp.tile([128, CT, HW + 1], bf16, tag="qb")
                nc.vector.tensor_copy(qb_s[:, :, :HW], qn_s[:])
                nc.vector.tensor_copy(
                    qb_s[:, :, HW:HW + 1],
                    ones_b[:, :, None].to_broadcast((128, CT, 1)))
                return qT_s, qb_s, qn_s

            with rep_ctx:
                xo = None
                nxt = load_batch(0)
                for b in range(BL):
                    qT_s, qb_s, qn_s = nxt
                    if b + 1 < BL:
                        nxt = load_batch(b + 1)

                    # ---- upper-triangle E (fp32r, bank-aligned chunks) + exp
                    # Rows j>=8 are short enough that a PAIR (2j', 2j'+1)
                    # shares one PSUM slot (second row at a bank-aligned
                    # base) and one paired 3D exp activation.
                    def e_mms(pe, j, base, dstart):
                        chunks = []
                        m = dstart
                        if (m // 256) % 2 == 1:
                            chunks.append((m, 256))
                            m += 256
                        while m < C:
                            chunks.append((m, 512))
                            m += 512
                        for m, w in chunks:
                            for k in range(2):
                                nc.tensor.matmul(
                                    pe[:, base + m - dstart:base + m - dstart + w],
                                    qT_s[:, k, 128 * j:128 * (j + 1)],
                                    qT_s[:, k, m:m + w],
                                    start=(k == 0),
                                    stop=(k == 1),
                                )

                    def mirror(j):
                        # mirror lower-triangle blocks fed by row-tile j
                        # (one batched 3D-out XBAR block-transpose)
                        j0 = 2 * (j // 2) + 2
                        if j0 < CT:
                            nc.sync.dma_start_transpose(
                                U[:, j0:CT, 128 * j:128 * (j + 1)],
                                U[:, j, 128 * j0:C],
                            )

                    for j in range(8):
                        dstart = 256 * (j // 2)
                        pe = psp.tile([128, C], f32, tag="ps")
                        e_mms(pe, j, dstart, dstart)
                        nc.scalar.activation(
                            U[:, j, dstart:], pe[:, dstart:], FT.Exp,
                            bias=shift_s[:], scale=-1.0,
                        )
                        mirror(j)
                    for jp in range(8, CT, 2):
                        dstart = 256 * (jp // 2)
                        w = C - dstart
                        off1 = dstart % 512
                        pe = psp.tile([128, C], f32, tag="ps")
                        e_mms(pe, jp, off1, dstart)
                        e_mms(pe, jp + 1, 1024 + off1, dstart)
                        pin = pe.rearrange(
                            "p (t h) -> p t h", t=2)[:, :, off1:off1 + w]
                        nc.scalar.activation(
                            U[:, jp:jp + 2, dstart:], pin, FT.Exp,
                            bias=shift_s[:], scale=-1.0,
                        )
                        mirror(jp)
                        mirror(jp + 1)

                    if b % 2 == 0:
                        xo = xop.tile([128, CT, 2 * HW], f32r, tag="xo")
                    off = (b % 2) * HW

                    # ---- O|Z = U @ [q|1]; high j first (they need no mirrors)
                    for j in range(CT - 1, -1, -1):
                        po = psp.tile([128, C], f32, tag="ps")
                        for i in range(CT):
                            nc.tensor.matmul(
                                po[:, :HW + 1],
                                U[:, i, 128 * j:128 * (j + 1)],
                                qb_s[:, i, :],
                                start=(i == 0),
                                stop=(i == CT - 1),
                            )
                        rg = zp.tile([128, 2], f32, tag="rg")
                        nc.vector.tensor_scalar_add(
                            rg[:, 0:1], po[:, HW:HW + 1], 1e-30)
                        nc.vector.reciprocal(rg[:, 0:1], rg[:, 0:1])
                        nc.vector.tensor_tensor(
                            rg[:, 1:2], rg[:, 0:1], gamma_s[:], ALU.mult)
                        nc.vector.tensor_scalar_mul(
                            xo[:, j, off:off + HW], po[:, :HW], rg[:, 1:2])
                        nc.vector.tensor_tensor(
                            xo[:, j, off:off + HW], xo[:, j, off:off + HW],
                            qn_s[:, j, :], ALU.add)

                    # ---- 1x1 conv on a pair of batches (free dim 392) ----
                    if b % 2 == 1:
                        b0 = b - 1
                        for t in range(OUT // 128):
                            pc = psp.tile([128, C], f32, tag="ps")
                            for i in range(CT):
                                nc.tensor.matmul(
                                    pc[:, :2 * HW],
                                    wT_s[:, i, 128 * t:128 * (t + 1)],
                                    xo[:, i, :],
                                    start=(i == 0),
                                    stop=(i == CT - 1),
                                )
                            y = yp.tile([128, 2, HW], f32, tag="y")
                            nc.vector.tensor_scalar_add(
                                y[:], pc[:, :2 * HW], bias_s[:, t:t + 1])
                            nc.scalar.dma_start(
                                outd[b0:b0 + 2, 128 * t:128 * (t + 1), :]
                                .rearrange("b p n -> p b n"),
                                y[:])

    nc.compile()
    return nc


def _get_nc():
    if "nc" not in _CACHE:
        _CACHE["nc"] = _build_nc()
    return _CACHE["nc"]


def _prep_in_maps(x, gamma, conv_w, conv_b):
    x = np.ascontiguousarray(np.asarray(x, dtype=np.float32))
    q = x.reshape(B, C, HW)
    W2 = np.asarray(conv_w, dtype=np.float32).reshape(OUT, C)
    wT = np.ascontiguousarray(W2.T).reshape(CT, 128, OUT)
    bias = np.ascontiguousarray(
        np.asarray(conv_b, dtype=np.float32).reshape(OUT // 128, 128).T)
    gc = np.full((128, 1), np.asarray(gamma, dtype=np.float32).reshape(-1)[0],
                 dtype=np.float32)

    in_maps = []
    for c in range(NCORES):
        qc = q[BL * c:BL * (c + 1)]              # [BL, C, HW]
        qtr = qc.transpose(0, 2, 1)              # [BL, HW, C]
        qT = np.zeros((BL, 2, 128, C), np.float32)
        qT[:, 0, :, :] = qtr[:, 0:128, :]
        qT[:, 1, 0:HW - 128, :] = qtr[:, 128:HW, :]
        qn = np.ascontiguousarray(qc.reshape(BL, CT, 128, HW))
        in_maps.append({
            "qT": qT, "qn": qn,
            "wT": wT, "bias": bias, "gammac": gc,
        })
    return in_maps


def run(x, gamma, conv_w, conv_b, trace=False, **kwargs):
    from concourse.bass_utils import run_bass_kernel_spmd

    nc = _get_nc()
    in_maps = _prep_in_maps(x, gamma, conv_w, conv_b)
    res = run_bass_kernel_spmd(nc, in_maps, core_ids=list(range(NCORES)),
                               trace=trace, **kwargs)
    outs = [np.asarray(res.results[i]["out"], dtype=np.float32)
            for i in range(NCORES)]
    full = np.concatenate(outs, axis=0).reshape(B, OUT, 14, 14)
    return full, res


def kernel(x, gamma, conv_w, conv_b):
    full, _ = run(x, gamma, conv_w, conv_b, trace=False)
    return full



# revision 30
# speedup vs baseline: 38.9066x; 38.9066x over previous
"""CAM (channel attention) module kernel for 8 TRN2 NeuronCores.

Reference computation (per batch b of 32, C=2048, N=H*W=196):
    E = q @ q.T                      # [C, C] channel energy
    A = softmax(rowmax(E) - E)       # == softmax(-E) row-wise (shift cancels)
    out = gamma * (A @ q) + x
    y = conv1x1(out, W) + bias       # 2048 -> 512

Sharding: pure data-parallel over batch, 4 batches per core, conv weights
replicated. No collectives.

Per-core kernel design (symmetric-E, PE kept ~100% busy in steady state):
  - E = q q^T is symmetric: compute only the block-upper-triangle — row-tile
    j covers columns [128*j, 2048) — in bank-contained 512/384/256/128-wide
    fp32r (FP22) matmul chunks, K padded 196 -> 2x128.
  - All PSUM flows through ONE pool of 4 x [128,1024] tiles (2 banks each,
    the full 8 banks): E rows split at the absolute col-1024 boundary into
    L/R tiles, O groups and conv each take one tile. Rotation depth 4 keeps
    tail tiles gated by long-drained slots instead of the previous drain.
  - U = exp(-E - 40) on ScalarE straight out of PSUM into bf16 (any
    constant shift cancels in the softmax normalization). E row-tiles are
    processed in DESCENDING order so mirror production matches the O loop's
    consumption order. Wide rows (j<8) split their exp: the right piece
    (graded per row, narrow for late rows) runs during the E phase; the
    left piece is staged to SBUF by VectorE and exp'd during the O phase,
    keeping ScalarE off the E->O critical path.
  - Lower-triangle U blocks mirror-filled with batched SBUF->SBUF DMA
    block-transposes (bf16 XBAR path) on the sync queue; input prefetch is
    issued after the mirrors so transposes never queue behind loads.
  - U symmetric => stored U tiles serve directly as lhsT for O = U @ [q|1]
    (bf16, free dim 197; the ones column yields the row sums Z). O groups
    accumulate i DESCENDING so the first group overlaps the last exps.
  - normalize + residual fused on VectorE: xo = po*(gamma/Z) + q (bf16).
  - per-batch 1x1 conv (bf16 weights, free dim 196) emitted at the next
    E->O boundary: pure-PE work that bridges the exp-tail bubble. Bias is
    added on the PSUM evacuation; output DMA issues from the GpSimd queue.
"""

import numpy as np

B = 32
NCORES = 8
BL = B // NCORES  # batches per core
C = 2048
HW = 196
OUT = 512
CT = C // 128  # 16 c-tiles
SHIFT = 40.0

# exp split position per single row j=0..7: right half [split, 2048) is exp'd
# during the E phase (graded narrow for late rows so the Act backlog at the
# E->O boundary is tiny); left half [dstart, split) is staged to SBUF and
# exp'd during the O phase.
_SPLIT = {7: 1024, 6: 1024, 5: 1280, 4: 1280, 3: 1536, 2: 1536, 1: 1664,
          0: 1792}
_STG_OFF = {}
_off = 0
for _j in range(8):
    _STG_OFF[_j] = _off
    _off += _SPLIT[_j] - 128 * _j
_STG_TOTAL = _off

_CACHE = {}


def _build_nc(reps=1, nbatches=BL):
    # nbatches > BL unrolls extra (modulo-indexed) batches for steady-state
    # timeline simulation only; production always uses nbatches=BL.
    import contextlib
    import concourse.bacc as bacc
    import concourse.tile as tile
    import concourse.mybir as mybir

    f32 = mybir.dt.float32
    f32r = mybir.dt.float32r
    bf16 = mybir.dt.bfloat16
    FT = mybir.ActivationFunctionType
    ALU = mybir.AluOpType

    nc = bacc.Bacc("TRN2", target_bir_lowering=False, debug=False,
                   num_devices=NCORES)

    qTd = nc.dram_tensor("qT", [BL, 2, 128, C], f32r, kind="ExternalInput")
    qbd = nc.dram_tensor("qb", [BL, 128, CT * (HW + 1)], bf16,
                         kind="ExternalInput")
    wTd = nc.dram_tensor("wT", [CT, 128, OUT], bf16, kind="ExternalInput")
    biasd = nc.dram_tensor("bias", [128, OUT // 128], f32, kind="ExternalInput")
    gammad = nc.dram_tensor("gammac", [128, 1], f32, kind="ExternalInput")
    outd = nc.dram_tensor("out", [max(nbatches, BL), OUT, HW], f32,
                          kind="ExternalOutput")

    with tile.TileContext(nc) as tc:
        with (
            tc.tile_pool(name="const", bufs=1) as constp,
            tc.tile_pool(name="qt", bufs=2) as qtp,
            tc.tile_pool(name="qbp", bufs=2) as qbp,
            tc.tile_pool(name="u", bufs=1) as up,
            tc.tile_pool(name="xo", bufs=2) as xop,
            tc.tile_pool(name="y", bufs=4) as yp,
            tc.tile_pool(name="z", bufs=8) as zp,
            tc.tile_pool(name="psum", bufs=4, space="PSUM") as psp,
        ):
            # conv weights: chunked on the Activation queue so batch-0 input
            # loads (SP queue) interleave instead of waiting behind one 4MB DMA
            stg = constp.tile([128, _STG_TOTAL], f32)  # exp-left staging
            wT_s = constp.tile([128, CT, OUT], bf16)
            for ci in range(0, CT, 2):
                nc.scalar.dma_start(
                    wT_s[:, ci:ci + 2],
                    wTd[ci:ci + 2].rearrange("i p o -> p i o"))
            bias_s = constp.tile([128, OUT // 128], f32)
            nc.scalar.dma_start(bias_s[:], biasd[:])
            gamma_s = constp.tile([128, 1], f32)
            nc.scalar.dma_start(gamma_s[:], gammad[:])
            shift_s = constp.tile([128, 1], f32)
            nc.vector.memset(shift_s[:], -SHIFT)

            U = up.tile([128, CT, C], bf16)

            rep_ctx = (
                tc.For_i(0, reps, 1,
                         hint_engines=tuple(mybir.EngineType))
                if reps > 1 else contextlib.nullcontext()
            )
            def load_batch(b):
                b = b % BL
                qT_s = qtp.tile([128, 2, C], f32r, tag="qt")
                nc.sync.dma_start(qT_s[:, 0], qTd[b, 0])
                nc.sync.dma_start(qT_s[:, 1], qTd[b, 1])
                # qb pre-converted host-side: partition-contiguous bf16 load
                qb_s = qbp.tile([128, CT, HW + 1], bf16, tag="qb")
                nc.sync.dma_start(
                    qb_s[:].rearrange("p i n -> p (i n)"), qbd[b])
                return qT_s, qb_s

            with rep_ctx:
                pend = None  # (batch_idx, xo_tile) awaiting its 1x1 conv
                nxt = load_batch(0)

                def conv_batch(bc, xo_c):
                    # per-batch 1x1 conv (F=196): pure PE work that fills
                    # the E->O boundary bubble while Act drains exp tails
                    for t in range(OUT // 128):
                        pc = psp.tile([128, 1024], f32, tag="ps")
                        for i in range(CT - 1, -1, -1):
                            nc.tensor.matmul(
                                pc[:, :HW],
                                wT_s[:, i, 128 * t:128 * (t + 1)],
                                xo_c[:, i, :],
                                start=(i == CT - 1),
                                stop=(i == 0),
                            )
                        y = yp.tile([128, HW], f32, tag="y")
                        nc.vector.tensor_scalar_add(
                            y[:], pc[:, :HW], bias_s[:, t:t + 1])
                        nc.gpsimd.dma_start(
                            outd[bc, 128 * t:128 * (t + 1), :], y[:])

                for b in range(nbatches):
                    qT_s, qb_s = nxt

                    # ---- upper-triangle E (fp32r, bank-aligned chunks) + exp
                    # Rows j>=8 are short enough that a PAIR (2j', 2j'+1)
                    # shares one PSUM slot (second row at a bank-aligned
                    # base) and one paired 3D exp activation.
                    def e_mms(pe, j, lo, hi, blk):
                        # chunks of row j covering absolute cols [lo, hi),
                        # written to tile `pe` whose base is absolute `blk`;
                        # each chunk stays inside one 512-col PSUM bank
                        chunks = []
                        m = lo
                        if m % 512:
                            w0 = min(512 - m % 512, hi - m)
                            chunks.append((m, w0))
                            m += w0
                        while m < hi:
                            chunks.append((m, 512))
                            m += 512
                        for m, w in chunks:
                            for k in range(2):
                                nc.tensor.matmul(
                                    pe[:, m - blk:m - blk + w],
                                    qT_s[:, k, 128 * j:128 * (j + 1)],
                                    qT_s[:, k, m:m + w],
                                    start=(k == 0),
                                    stop=(k == 1),
                                )

                    def mirror(j):
                        # mirror lower-triangle blocks fed by row-tile j
                        # (one batched 3D-out XBAR block-transpose)
                        j0 = j + 1
                        if j0 < CT:
                            nc.sync.dma_start_transpose(
                                U[:, j0:CT, 128 * j:128 * (j + 1)],
                                U[:, j, 128 * j0:C],
                            )

                    # Row order DESCENDING so mirror production order matches
                    # the O loop's (j=15..0) consumption order: no O group
                    # ever waits on a late mirror transpose.
                    for jp in range(CT - 1, 7, -1):
                        dstart = 128 * jp
                        pe = psp.tile([128, 1024], f32, tag="ps")
                        e_mms(pe, jp, dstart, C, 1024)
                        nc.scalar.activation(
                            U[:, jp, dstart:], pe[:, dstart - 1024:], FT.Exp,
                            bias=shift_s[:], scale=-1.0,
                        )
                        mirror(jp)
                    # Wide single rows (j=7..0): split the exp at col 1024.
                    # Right halves exp'd from PSUM now (all that O[15..8]
                    # needs); left halves staged to SBUF by DVE and exp'd
                    # during the O phase, taking Act off the critical path.
                    for j in range(7, -1, -1):
                        dstart = 128 * j
                        sp = _SPLIT[j]
                        peL = psp.tile([128, 1024], f32, tag="ps")
                        peR = psp.tile([128, 1024], f32, tag="ps")
                        e_mms(peL, j, dstart, 1024, 0)
                        e_mms(peR, j, 1024, C, 1024)
                        # stage [dstart:sp) to SBUF: piece from L, and from R
                        # when the split lies right of the 1024 boundary
                        nc.vector.tensor_copy(
                            stg[:, _STG_OFF[j]:_STG_OFF[j] + 1024 - dstart],
                            peL[:, dstart:])
                        if sp > 1024:
                            nc.vector.tensor_copy(
                                stg[:, _STG_OFF[j] + 1024 - dstart:
                                    _STG_OFF[j] + sp - dstart],
                                peR[:, :sp - 1024])
                        nc.scalar.activation(
                            U[:, j, sp:], peR[:, sp - 1024:], FT.Exp,
                            bias=shift_s[:], scale=-1.0,
                        )
                    # deferred left-half exps (Act runs these during O),
                    # widest-covering first to meet O-group deadlines
                    for j in sorted(range(8), key=lambda j: -_SPLIT[j]):
                        dstart = 128 * j
                        sp = _SPLIT[j]
                        nc.scalar.activation(
                            U[:, j, dstart:sp],
                            stg[:, _STG_OFF[j]:_STG_OFF[j] + sp - dstart],
                            FT.Exp, bias=shift_s[:], scale=-1.0,
                        )
                    # single-row mirrors (need both exp halves for j<=5)
                    for j in range(7, -1, -1):
                        mirror(j)

                    # previous batch's conv fills the E->O bubble here
                    if pend is not None:
                        conv_batch(*pend)
                        pend = None
                    xo = xop.tile([128, CT, HW], bf16, tag="xo")

                    # prefetch next batch AFTER the E phase so this batch's
                    # mirror transposes aren't queued behind the loads
                    if b + 1 < nbatches:
                        nxt = load_batch(b + 1)

                    # ---- O|Z = U @ [q|1]; high j first (they need no mirrors)
                    for j in range(CT - 1, -1, -1):
                        po = psp.tile([128, 1024], f32, tag="ps")
                        for i in range(CT - 1, -1, -1):
                            nc.tensor.matmul(
                                po[:, :HW + 1],
                                U[:, i, 128 * j:128 * (j + 1)],
                                qb_s[:, i, :],
                                start=(i == CT - 1),
                                stop=(i == 0),
                            )
                        rg = zp.tile([128, 2], f32, tag="rg")
                        nc.vector.tensor_scalar_add(
                            rg[:, 0:1], po[:, HW:HW + 1], 1e-30)
                        nc.vector.reciprocal(rg[:, 0:1], rg[:, 0:1])
                        nc.vector.tensor_tensor(
                            rg[:, 1:2], rg[:, 0:1], gamma_s[:], ALU.mult)
                        nc.vector.scalar_tensor_tensor(
                            xo[:, j, :], po[:, :HW], rg[:, 1:2],
                            qb_s[:, j, :HW], ALU.mult, ALU.add)

                    pend = (b, xo)
                # drain: conv of the final batch (rep-boundary filler)
                conv_batch(*pend)

    nc.compile()
    return nc


def _get_nc():
    if "nc" not in _CACHE:
        _CACHE["nc"] = _build_nc()
    return _CACHE["nc"]


def _prep_in_maps(x, gamma, conv_w, conv_b):
    x = np.ascontiguousarray(np.asarray(x, dtype=np.float32))
    q = x.reshape(B, C, HW)
    W2 = np.asarray(conv_w, dtype=np.float32).reshape(OUT, C)
    import ml_dtypes
    wT = np.ascontiguousarray(W2.T).reshape(CT, 128, OUT).astype(ml_dtypes.bfloat16)
    bias = np.ascontiguousarray(
        np.asarray(conv_b, dtype=np.float32).reshape(OUT // 128, 128).T)
    gc = np.full((128, 1), np.asarray(gamma, dtype=np.float32).reshape(-1)[0],
                 dtype=np.float32)

    import ml_dtypes
    in_maps = []
    for c in range(NCORES):
        qc = q[BL * c:BL * (c + 1)]              # [BL, C, HW]
        qtr = qc.transpose(0, 2, 1)              # [BL, HW, C]
        qT = np.zeros((BL, 2, 128, C), np.float32)
        qT[:, 0, :, :] = qtr[:, 0:128, :]
        qT[:, 1, 0:HW - 128, :] = qtr[:, 128:HW, :]
        qn = np.ascontiguousarray(qc.reshape(BL, CT, 128, HW))
        # [BL, 128, CT, HW+1]: qb[b, p, i, :HW] = q row 128i+p, last col = 1
        qb4 = np.ones((BL, 128, CT, HW + 1), ml_dtypes.bfloat16)
        qb4[:, :, :, :HW] = qn.transpose(0, 2, 1, 3).astype(ml_dtypes.bfloat16)
        qb = qb4.reshape(BL, 128, CT * (HW + 1))
        in_maps.append({
            "qT": qT, "qb": qb,
            "wT": wT, "bias": bias, "gammac": gc,
        })
    return in_maps


def run(x, gamma, conv_w, conv_b, trace=False, **kwargs):
    from concourse.bass_utils import run_bass_kernel_spmd

    nc = _get_nc()
    in_maps = _prep_in_maps(x, gamma, conv_w, conv_b)
    res = run_bass_kernel_spmd(nc, in_maps, core_ids=list(range(NCORES)),
                               trace=trace, **kwargs)
    outs = [np.asarray(res.results[i]["out"], dtype=np.float32)
            for i in range(NCORES)]
    full = np.concatenate(outs, axis=0).reshape(B, OUT, 14, 14)
    return full, res


def kernel(x, gamma, conv_w, conv_b):
    full, _ = run(x, gamma, conv_w, conv_b, trace=False)
    return full



# revision 32
# speedup vs baseline: 39.9270x; 1.0262x over previous
"""CAM (channel attention) module kernel for 8 TRN2 NeuronCores.

Reference computation (per batch b of 32, C=2048, N=H*W=196):
    E = q @ q.T                      # [C, C] channel energy
    A = softmax(rowmax(E) - E)       # == softmax(-E) row-wise (shift cancels)
    out = gamma * (A @ q) + x
    y = conv1x1(out, W) + bias       # 2048 -> 512

Sharding: pure data-parallel over batch, 4 batches per core, conv weights
replicated. No collectives.

Per-core kernel design (symmetric-E, PE kept ~100% busy in steady state):
  - E = q q^T is symmetric: compute only the block-upper-triangle — row-tile
    j covers columns [128*j, 2048) — in bank-contained 512/384/256/128-wide
    fp32r (FP22) matmul chunks, K padded 196 -> 2x128. The K sweep is OUTER
    (all chunks for k=0, then k=1) so each stationary operand is loaded once
    per sweep: fp32r disables fast-weight-load, so fewer LDWEIGHTS is a
    measured ~2% HW win (and fp32r E beats bf16 E on HW despite FWL).
  - All PSUM flows through ONE pool of 4 x [128,1024] tiles (2 banks each,
    the full 8 banks): E rows split at the absolute col-1024 boundary into
    L/R tiles, O groups and conv each take one tile. Rotation depth 4 keeps
    tail tiles gated by long-drained slots instead of the previous drain.
  - U = exp(-E - 40) on ScalarE straight out of PSUM into bf16 (any
    constant shift cancels in the softmax normalization). E row-tiles are
    processed in DESCENDING order so mirror production matches the O loop's
    consumption order. Wide rows (j<8) split their exp: the right piece
    (graded per row, narrow for late rows) runs during the E phase; the
    left piece is staged to SBUF by VectorE and exp'd during the O phase,
    keeping ScalarE off the E->O critical path.
  - Lower-triangle U blocks mirror-filled with batched SBUF->SBUF DMA
    block-transposes (bf16 XBAR path) on the sync queue; input prefetch is
    issued after the mirrors so transposes never queue behind loads.
  - U symmetric => stored U tiles serve directly as lhsT for O = U @ [q|1]
    (bf16, free dim 197; the ones column yields the row sums Z). O groups
    accumulate i DESCENDING so the first group overlaps the last exps.
  - normalize + residual fused on VectorE: xo = po*(gamma/Z) + q (bf16).
  - per-batch 1x1 conv (bf16 weights, free dim 196) emitted at the next
    E->O boundary: pure-PE work that bridges the exp-tail bubble. Bias is
    added on the PSUM evacuation; output DMA issues from the GpSimd queue.
"""

import numpy as np

B = 32
NCORES = 8
BL = B // NCORES  # batches per core
C = 2048
HW = 196
OUT = 512
CT = C // 128  # 16 c-tiles
SHIFT = 40.0

# exp split position per single row j=0..7: right half [split, 2048) is exp'd
# during the E phase (graded narrow for late rows so the Act backlog at the
# E->O boundary is tiny); left half [dstart, split) is staged to SBUF and
# exp'd during the O phase.
_SPLIT = {7: 1024, 6: 1024, 5: 1280, 4: 1280, 3: 1536, 2: 1536, 1: 1664,
          0: 1792}
_STG_OFF = {}
_off = 0
for _j in range(8):
    _STG_OFF[_j] = _off
    _off += _SPLIT[_j] - 128 * _j
_STG_TOTAL = _off

_CACHE = {}


def _build_nc(reps=1, nbatches=BL):
    # nbatches > BL unrolls extra (modulo-indexed) batches for steady-state
    # timeline simulation only; production always uses nbatches=BL.
    import contextlib
    import concourse.bacc as bacc
    import concourse.tile as tile
    import concourse.mybir as mybir

    f32 = mybir.dt.float32
    f32r = mybir.dt.float32r
    bf16 = mybir.dt.bfloat16
    FT = mybir.ActivationFunctionType
    ALU = mybir.AluOpType

    nc = bacc.Bacc("TRN2", target_bir_lowering=False, debug=False,
                   num_devices=NCORES)

    qTd = nc.dram_tensor("qT", [BL, 2, 128, C], f32r, kind="ExternalInput")
    qbd = nc.dram_tensor("qb", [BL, 128, CT * (HW + 1)], bf16,
                         kind="ExternalInput")
    wTd = nc.dram_tensor("wT", [CT, 128, OUT], bf16, kind="ExternalInput")
    biasd = nc.dram_tensor("bias", [128, OUT // 128], f32, kind="ExternalInput")
    gammad = nc.dram_tensor("gammac", [128, 1], f32, kind="ExternalInput")
    outd = nc.dram_tensor("out", [max(nbatches, BL), OUT, HW], f32,
                          kind="ExternalOutput")

    with tile.TileContext(nc) as tc:
        with (
            tc.tile_pool(name="const", bufs=1) as constp,
            tc.tile_pool(name="qt", bufs=2) as qtp,
            tc.tile_pool(name="qbp", bufs=2) as qbp,
            tc.tile_pool(name="u", bufs=1) as up,
            tc.tile_pool(name="xo", bufs=2) as xop,
            tc.tile_pool(name="y", bufs=4) as yp,
            tc.tile_pool(name="z", bufs=8) as zp,
            tc.tile_pool(name="psum", bufs=4, space="PSUM") as psp,
        ):
            # conv weights: chunked on the Activation queue so batch-0 input
            # loads (SP queue) interleave instead of waiting behind one 4MB DMA
            stg = constp.tile([128, _STG_TOTAL], f32)  # exp-left staging
            wT_s = constp.tile([128, CT, OUT], bf16)
            for ci in range(0, CT, 2):
                nc.scalar.dma_start(
                    wT_s[:, ci:ci + 2],
                    wTd[ci:ci + 2].rearrange("i p o -> p i o"))
            bias_s = constp.tile([128, OUT // 128], f32)
            nc.scalar.dma_start(bias_s[:], biasd[:])
            gamma_s = constp.tile([128, 1], f32)
            nc.scalar.dma_start(gamma_s[:], gammad[:])
            shift_s = constp.tile([128, 1], f32)
            nc.vector.memset(shift_s[:], -SHIFT)

            U = up.tile([128, CT, C], bf16)

            rep_ctx = (
                tc.For_i(0, reps, 1,
                         hint_engines=tuple(mybir.EngineType))
                if reps > 1 else contextlib.nullcontext()
            )
            def load_batch(b):
                b = b % BL
                qT_s = qtp.tile([128, 2, C], f32r, tag="qt")
                nc.sync.dma_start(qT_s[:, 0], qTd[b, 0])
                nc.sync.dma_start(qT_s[:, 1], qTd[b, 1])
                # qb pre-converted host-side: partition-contiguous bf16 load
                qb_s = qbp.tile([128, CT, HW + 1], bf16, tag="qb")
                nc.sync.dma_start(
                    qb_s[:].rearrange("p i n -> p (i n)"), qbd[b])
                return qT_s, qb_s

            with rep_ctx:
                pend = None  # (batch_idx, xo_tile) awaiting its 1x1 conv
                nxt = load_batch(0)

                def conv_batch(bc, xo_c):
                    # per-batch 1x1 conv (F=196): pure PE work that fills
                    # the E->O boundary bubble while Act drains exp tails
                    for t in range(OUT // 128):
                        pc = psp.tile([128, 1024], f32, tag="ps")
                        for i in range(CT - 1, -1, -1):
                            nc.tensor.matmul(
                                pc[:, :HW],
                                wT_s[:, i, 128 * t:128 * (t + 1)],
                                xo_c[:, i, :],
                                start=(i == CT - 1),
                                stop=(i == 0),
                            )
                        y = yp.tile([128, HW], f32, tag="y")
                        nc.vector.tensor_scalar_add(
                            y[:], pc[:, :HW], bias_s[:, t:t + 1])
                        nc.gpsimd.dma_start(
                            outd[bc, 128 * t:128 * (t + 1), :], y[:])

                for b in range(nbatches):
                    qT_s, qb_s = nxt

                    # ---- upper-triangle E (fp32r, bank-aligned chunks) + exp
                    # Rows j>=8 are short enough that a PAIR (2j', 2j'+1)
                    # shares one PSUM slot (second row at a bank-aligned
                    # base) and one paired 3D exp activation.
                    def e_mms(pe, j, lo, hi, blk):
                        # chunks of row j covering absolute cols [lo, hi),
                        # written to tile `pe` whose base is absolute `blk`;
                        # each chunk stays inside one 512-col PSUM bank
                        chunks = []
                        m = lo
                        if m % 512:
                            w0 = min(512 - m % 512, hi - m)
                            chunks.append((m, w0))
                            m += w0
                        while m < hi:
                            chunks.append((m, 512))
                            m += 512
                        for k in range(2):
                            for m, w in chunks:
                                nc.tensor.matmul(
                                    pe[:, m - blk:m - blk + w],
                                    qT_s[:, k, 128 * j:128 * (j + 1)],
                                    qT_s[:, k, m:m + w],
                                    start=(k == 0),
                                    stop=(k == 1),
                                )

                    def mirror(j):
                        # mirror lower-triangle blocks fed by row-tile j
                        # (one batched 3D-out XBAR block-transpose)
                        j0 = j + 1
                        if j0 < CT:
                            nc.sync.dma_start_transpose(
                                U[:, j0:CT, 128 * j:128 * (j + 1)],
                                U[:, j, 128 * j0:C],
                            )

                    # Row order DESCENDING so mirror production order matches
                    # the O loop's (j=15..0) consumption order: no O group
                    # ever waits on a late mirror transpose.
                    for jp in range(CT - 1, 7, -1):
                        dstart = 128 * jp
                        pe = psp.tile([128, 1024], f32, tag="ps")
                        e_mms(pe, jp, dstart, C, 1024)
                        nc.scalar.activation(
                            U[:, jp, dstart:], pe[:, dstart - 1024:], FT.Exp,
                            bias=shift_s[:], scale=-1.0,
                        )
                        mirror(jp)
                    # Wide single rows (j=7..0): split the exp at col 1024.
                    # Right halves exp'd from PSUM now (all that O[15..8]
                    # needs); left halves staged to SBUF by DVE and exp'd
                    # during the O phase, taking Act off the critical path.
                    for j in range(7, -1, -1):
                        dstart = 128 * j
                        sp = _SPLIT[j]
                        peL = psp.tile([128, 1024], f32, tag="ps")
                        peR = psp.tile([128, 1024], f32, tag="ps")
                        e_mms(peL, j, dstart, 1024, 0)
                        e_mms(peR, j, 1024, C, 1024)
                        # stage [dstart:sp) to SBUF: piece from L, and from R
                        # when the split lies right of the 1024 boundary
                        nc.vector.tensor_copy(
                            stg[:, _STG_OFF[j]:_STG_OFF[j] + 1024 - dstart],
                            peL[:, dstart:])
                        if sp > 1024:
                            nc.vector.tensor_copy(
                                stg[:, _STG_OFF[j] + 1024 - dstart:
                                    _STG_OFF[j] + sp - dstart],
                                peR[:, :sp - 1024])
                        nc.scalar.activation(
                            U[:, j, sp:], peR[:, sp - 1024:], FT.Exp,
                            bias=shift_s[:], scale=-1.0,
                        )
                    # deferred left-half exps (Act runs these during O),
                    # widest-covering first to meet O-group deadlines
                    for j in sorted(range(8), key=lambda j: -_SPLIT[j]):
                        dstart = 128 * j
                        sp = _SPLIT[j]
                        nc.scalar.activation(
                            U[:, j, dstart:sp],
                            stg[:, _STG_OFF[j]:_STG_OFF[j] + sp - dstart],
                            FT.Exp, bias=shift_s[:], scale=-1.0,
                        )
                    # single-row mirrors (need both exp halves for j<=5)
                    for j in range(7, -1, -1):
                        mirror(j)

                    # previous batch's conv fills the E->O bubble here
                    if pend is not None:
                        conv_batch(*pend)
                        pend = None
                    xo = xop.tile([128, CT, HW], bf16, tag="xo")

                    # prefetch next batch AFTER the E phase so this batch's
                    # mirror transposes aren't queued behind the loads
                    if b + 1 < nbatches:
                        nxt = load_batch(b + 1)

                    # ---- O|Z = U @ [q|1]; high j first (they need no mirrors)
                    for j in range(CT - 1, -1, -1):
                        po = psp.tile([128, 1024], f32, tag="ps")
                        for i in range(CT - 1, -1, -1):
                            nc.tensor.matmul(
                                po[:, :HW + 1],
                                U[:, i, 128 * j:128 * (j + 1)],
                                qb_s[:, i, :],
                                start=(i == CT - 1),
                                stop=(i == 0),
                            )
                        rg = zp.tile([128, 2], f32, tag="rg")
                        nc.vector.tensor_scalar_add(
                            rg[:, 0:1], po[:, HW:HW + 1], 1e-30)
                        nc.vector.reciprocal(rg[:, 0:1], rg[:, 0:1])
                        nc.vector.tensor_tensor(
                            rg[:, 1:2], rg[:, 0:1], gamma_s[:], ALU.mult)
                        nc.vector.scalar_tensor_tensor(
                            xo[:, j, :], po[:, :HW], rg[:, 1:2],
                            qb_s[:, j, :HW], ALU.mult, ALU.add)

                    pend = (b, xo)
                # drain: conv of the final batch (rep-boundary filler)
                conv_batch(*pend)

    nc.compile()
    return nc


def _get_nc():
    if "nc" not in _CACHE:
        _CACHE["nc"] = _build_nc()
    return _CACHE["nc"]


def _prep_in_maps(x, gamma, conv_w, conv_b):
    x = np.ascontiguousarray(np.asarray(x, dtype=np.float32))
    q = x.reshape(B, C, HW)
    W2 = np.asarray(conv_w, dtype=np.float32).reshape(OUT, C)
    import ml_dtypes
    wT = np.ascontiguousarray(W2.T).reshape(CT, 128, OUT).astype(ml_dtypes.bfloat16)
    bias = np.ascontiguousarray(
        np.asarray(conv_b, dtype=np.float32).reshape(OUT // 128, 128).T)
    gc = np.full((128, 1), np.asarray(gamma, dtype=np.float32).reshape(-1)[0],
                 dtype=np.float32)

    import ml_dtypes
    in_maps = []
    for c in range(NCORES):
        qc = q[BL * c:BL * (c + 1)]              # [BL, C, HW]
        qtr = qc.transpose(0, 2, 1)              # [BL, HW, C]
        qT = np.zeros((BL, 2, 128, C), np.float32)
        qT[:, 0, :, :] = qtr[:, 0:128, :]
        qT[:, 1, 0:HW - 128, :] = qtr[:, 128:HW, :]
        qn = np.ascontiguousarray(qc.reshape(BL, CT, 128, HW))
        # [BL, 128, CT, HW+1]: qb[b, p, i, :HW] = q row 128i+p, last col = 1
        qb4 = np.ones((BL, 128, CT, HW + 1), ml_dtypes.bfloat16)
        qb4[:, :, :, :HW] = qn.transpose(0, 2, 1, 3).astype(ml_dtypes.bfloat16)
        qb = qb4.reshape(BL, 128, CT * (HW + 1))
        in_maps.append({
            "qT": qT, "qb": qb,
            "wT": wT, "bias": bias, "gammac": gc,
        })
    return in_maps


def run(x, gamma, conv_w, conv_b, trace=False, **kwargs):
    from concourse.bass_utils import run_bass_kernel_spmd

    nc = _get_nc()
    in_maps = _prep_in_maps(x, gamma, conv_w, conv_b)
    res = run_bass_kernel_spmd(nc, in_maps, core_ids=list(range(NCORES)),
                               trace=trace, **kwargs)
    outs = [np.asarray(res.results[i]["out"], dtype=np.float32)
            for i in range(NCORES)]
    full = np.concatenate(outs, axis=0).reshape(B, OUT, 14, 14)
    return full, res


def kernel(x, gamma, conv_w, conv_b):
    full, _ = run(x, gamma, conv_w, conv_b, trace=False)
    return full

